# revision 11
# baseline (speedup 1.0000x reference)
"""Trainium2 Bass kernel for nn_Aggregator (retrieval_knn).

Reference computation: for each of B*T*Ro*S = 524288 query points, find the
8 nearest of 512 keypoints (per batch), threshold at R=0.12, cap at 48 valid
points per ray (64 points), emit (neighbor_idx, shading_pts, neighbor_dist,
mask).

Device part (per core, SPMD over 8 cores; core = (batch, ray-half)):
  - PE matmul computes s[q,m] = 2*q.k - |k|^2  (= a2[q] - d2[q,m]) for a
    128-query subtile against C candidate keypoints.
  - DVE max/max_index extract the top-8 values (= 8 smallest d2) + indices.
  - d2 = a2 - val, clamped, sqrt -> 8 ascending distances per query.
Host part: radius mask, per-ray cumsum cap, -1/0 fills, mask construction
(cheap O(N) numpy, exactly mirroring the reference semantics).
"""

import os
import sys

import numpy as np

sys.path.insert(0, "/opt/trn_rl_repo")

from contextlib import ExitStack

import concourse.bass as bass
import concourse.tile as tile
from concourse import bacc, mybir
from concourse.bass_utils import run_bass_kernel_spmd

# Problem constants
B, T, RO, S, _D = 4, 2, 1024, 64, 3
NKP = 512
K = 8
R = 0.12
MAX_SHADING_PTS = 48

N_CORES = 8
NQ_CORE = (B * T * RO * S) // N_CORES  # 65536 queries per core

F32 = mybir.dt.float32
F32R = mybir.dt.float32r
U32 = mybir.dt.uint32


def build_program(nsub, c, nrhs, g, use_f32r=False, reps=1):
    """Build the Bass program.

    nsub: number of 128-query subtiles per core
    c:    candidate keypoints per subtile
    nrhs: number of rhs matrices (1 = shared by all subtiles, else nsub)
    g:    subtiles per output-staging group (nsub % g == 0)
    reps: repeat the whole computation (timing only)
    """
    assert nsub % g == 0
    n = nsub * 128
    nc = bacc.Bacc("TRN2", target_bir_lowering=False)
    lhs = nc.declare_dram_parameter("lhs", [4, n], F32, isOutput=False)
    rhs = nc.declare_dram_parameter("rhs", [nrhs, 4, c], F32, isOutput=False)
    a2d = nc.declare_dram_parameter("a2d", [nsub // g, 128, g], F32, isOutput=False)
    odist = nc.declare_dram_parameter("odist", [n, 8], F32, isOutput=True)
    oidx = nc.declare_dram_parameter("oidx", [n, 8], U32, isOutput=True)

    lc_sub = min(8, g)  # subtiles per lhs/rhs load chunk
    assert g % lc_sub == 0

    with tile.TileContext(nc) as tc, ExitStack() as ctx:
        if reps > 1:
            ctx.enter_context(tc.For_i(0, reps, 1))
        lpool = ctx.enter_context(tc.tile_pool(name="lhs", bufs=3))
        rpool = ctx.enter_context(tc.tile_pool(name="rhs", bufs=3))
        ppool = ctx.enter_context(tc.tile_pool(name="psum", bufs=6, space="PSUM"))
        dpool = ctx.enter_context(tc.tile_pool(name="d2", bufs=4))
        wpool = ctx.enter_context(tc.tile_pool(name="wide", bufs=2))

        rall = None
        if nrhs == 1:
            rall = rpool.tile([4, c], F32)
            nc.sync.dma_start(out=rall[:], in_=rhs[0])

        lc = None
        rc = None
        for m in range(nsub // g):
            wv = wpool.tile([128, g * 8], F32)
            wi = wpool.tile([128, g * 8], U32)
            a2t = wpool.tile([128, g], F32)
            nc.sync.dma_start(out=a2t[:], in_=a2d[m])
            for j in range(g):
                s = m * g + j
                if s % lc_sub == 0:
                    lc = lpool.tile([4, lc_sub * 128], F32)
                    nc.sync.dma_start(
                        out=lc[:], in_=lhs[:, s * 128 : (s + lc_sub) * 128]
                    )
                    if nrhs > 1:
                        rc = rpool.tile([4, lc_sub * c], F32)
                        nc.sync.dma_start(
                            out=rc[:],
                            in_=rhs[s : s + lc_sub].rearrange("s f c -> f (s c)"),
                        )
                jj = s % lc_sub
                lslice = lc[:, jj * 128 : (jj + 1) * 128]
                rslice = rall[:] if nrhs == 1 else rc[:, jj * c : (jj + 1) * c]
                if use_f32r:
                    lslice = lslice.bitcast(F32R)
                    rslice = rslice.bitcast(F32R)
                psum = ppool.tile([128, c], F32)
                nc.tensor.matmul(
                    psum[:], lhsT=lslice, rhs=rslice, start=True, stop=True
                )
                d2t = dpool.tile([128, c], F32)
                nc.scalar.copy(out=d2t[:], in_=psum[:])
                nc.vector.max(wv[:, j * 8 : (j + 1) * 8], d2t[:])
                nc.vector.max_index(
                    wi[:, j * 8 : (j + 1) * 8], wv[:, j * 8 : (j + 1) * 8], d2t[:]
                )
            # d2 = a2 - val (clamped to >= 1e-12), dist = sqrt(d2)
            a2x = wpool.tile([128, g * 8], F32)
            a2x3 = a2x[:].rearrange("p (g e) -> p g e", e=8)
            for e in range(8):
                nc.vector.tensor_copy(a2x3[:, :, e], a2t[:])
            d2n = wpool.tile([128, g * 8], F32)
            # d2n = val - a2 = -(d2);  min with -1e-12  ==  -(max(d2, 1e-12))
            nc.vector.tensor_sub(d2n[:], wv[:], a2x[:])
            nc.vector.tensor_scalar_min(d2n[:], d2n[:], -1e-12)
            dst = wpool.tile([128, g * 8], F32)
            nc.scalar.activation(
                dst[:], d2n[:], func=mybir.ActivationFunctionType.Sqrt, scale=-1.0
            )
            orows = odist[m * g * 128 : (m + 1) * g * 128].rearrange(
                "(p g) e -> p (g e)", p=128
            )
            nc.sync.dma_start(out=orows, in_=dst[:])
            irows = oidx[m * g * 128 : (m + 1) * g * 128].rearrange(
                "(p g) e -> p (g e)", p=128
            )
            nc.sync.dma_start(out=irows, in_=wi[:])
    nc.compile()
    return nc


# ---------------------------------------------------------------------------
# Host-side orchestration
# ---------------------------------------------------------------------------

_PROG_CACHE = {}


def _get_program(nsub, c, nrhs, g):
    key = (nsub, c, nrhs, g)
    if key not in _PROG_CACHE:
        _PROG_CACHE[key] = build_program(nsub, c, nrhs, g)
    return _PROG_CACHE[key]


def _core_inputs_v1(q, kp):
    """Simple dense config: one rhs of all 512 keypoints, identity order.

    q:  [65536, 3] float32 queries of this core
    kp: [512, 3] float32 keypoints of this core's batch
    Returns in_map dict. Device row r maps to query  (s*128 + p)  with
    s = (r // (128*g))*g + r % g,  p = (r // g) % 128.
    """
    nsub, c, g = NQ_CORE // 128, NKP, 64
    lhs = np.empty((4, NQ_CORE), np.float32)
    lhs[:3] = q.T
    lhs[3] = 1.0
    b2 = (kp[:, 0] * kp[:, 0] + kp[:, 1] * kp[:, 1]) + kp[:, 2] * kp[:, 2]
    rhs = np.empty((1, 4, c), np.float32)
    rhs[0, :3] = 2.0 * kp.T
    rhs[0, 3] = -b2
    a2 = (q[:, 0] * q[:, 0] + q[:, 1] * q[:, 1]) + q[:, 2] * q[:, 2]
    a2d = np.ascontiguousarray(
        a2.reshape(nsub // g, g, 128).transpose(0, 2, 1)
    )
    return {"lhs": lhs, "rhs": rhs, "a2d": a2d}


def _devrow_to_query(nsub, g):
    """origq[r] for device output row r (v1 ordering)."""
    r = np.arange(nsub * 128)
    m = r // (128 * g)
    j = r % g
    p = (r // g) % 128
    return (m * g + j) * 128 + p


def kernel(x, kp_pos):
    x = np.asarray(x, dtype=np.float32)
    kp_pos = np.asarray(kp_pos, dtype=np.float32)
    rays = T * RO

    nsub, c, nrhs, g = NQ_CORE // 128, NKP, 1, 64
    nc = _get_program(nsub, c, nrhs, g)

    xq = x.reshape(B, 2, rays // 2 * S, 3)
    in_maps = []
    for core in range(N_CORES):
        b, half = divmod(core, 2)
        in_maps.append(_core_inputs_v1(xq[b, half], kp_pos[b]))

    res = run_bass_kernel_spmd(nc, in_maps, core_ids=list(range(N_CORES)))

    perm = _devrow_to_query(nsub, g)
    inv = np.empty_like(perm)
    inv[perm] = np.arange(perm.size)

    vals = np.empty((B, rays * S, 8), np.float32)
    idx = np.empty((B, rays * S, 8), np.int32)
    half_n = rays // 2 * S
    for core in range(N_CORES):
        b, half = divmod(core, 2)
        od = res.results[core]["odist"]
        oi = res.results[core]["oidx"].view(np.int32)
        vals[b, half * half_n : (half + 1) * half_n] = od[inv]
        idx[b, half * half_n : (half + 1) * half_n] = oi[inv]

    return _postprocess(x, vals, idx)


def _postprocess(x, vals, idx):
    """vals: [B, rays*S, 8] ascending distances; idx: keypoint ids (0..511)."""
    rays = T * RO
    vals = vals.reshape(B, rays, S, 8)
    idx = idx.reshape(B, rays, S, 8)

    valid_nb = vals < R
    offset = (NKP * np.arange(B, dtype=np.int32)).reshape(B, 1, 1, 1)
    nb_idx = np.where(valid_nb, idx + offset, -1).astype(np.int32)

    valid_pts = valid_nb[..., 0:1]  # any() == slot 0 since ascending
    csum = np.cumsum(valid_pts.astype(np.int32), axis=-2)
    valid_pts = np.logical_and(valid_pts, csum <= MAX_SHADING_PTS)

    nb_idx = np.where(valid_pts, nb_idx, -1)
    nb_dist = np.where(np.logical_and(valid_pts, valid_nb), vals, 0.0).astype(
        np.float32
    )
    shading = np.where(valid_pts, x.reshape(B, rays, S, 3), 0.0).astype(np.float32)

    num_valid = valid_pts.sum(axis=-2, keepdims=True)
    mask = np.arange(MAX_SHADING_PTS).reshape(1, 1, -1, 1) < num_valid

    return (
        nb_idx.reshape(B, T, RO, S, K),
        shading.reshape(B, T, RO, S, 3),
        nb_dist.reshape(B, T, RO, S, K),
        mask.reshape(B, T, RO, MAX_SHADING_PTS, 1),
    )


# revision 13
# speedup vs baseline: 2.5212x; 2.5212x over previous
"""Trainium2 Bass kernel for nn_Aggregator (retrieval_knn).

Reference computation: for each of B*T*Ro*S = 524288 query points, find the
8 nearest of 512 keypoints (per batch), threshold at R=0.12, cap at 48 valid
points per ray (64 points), emit (neighbor_idx, shading_pts, neighbor_dist,
mask).

Device part (per core, SPMD over 8 cores; core = (batch, ray-half)):
  - PE matmul computes s[q,m] = 2*q.k - |k|^2  (= a2[q] - d2[q,m]) for a
    128-query subtile against C candidate keypoints.
  - DVE max/max_index extract the top-8 values (= 8 smallest d2) + indices.
  - d2 = a2 - val, clamped, sqrt -> 8 ascending distances per query.
Host part: radius mask, per-ray cumsum cap, -1/0 fills, mask construction
(cheap O(N) numpy, exactly mirroring the reference semantics).
"""

import os
import sys

import numpy as np

sys.path.insert(0, "/opt/trn_rl_repo")

from contextlib import ExitStack

import concourse.bass as bass
import concourse.tile as tile
from concourse import bacc, mybir
from concourse.bass_utils import run_bass_kernel_spmd

# Problem constants
B, T, RO, S, _D = 4, 2, 1024, 64, 3
NKP = 512
K = 8
R = 0.12
MAX_SHADING_PTS = 48

N_CORES = 8
NQ_CORE = (B * T * RO * S) // N_CORES  # 65536 queries per core

F32 = mybir.dt.float32
F32R = mybir.dt.float32r
U32 = mybir.dt.uint32


def build_program(nsub, c, nrhs, g, use_f32r=False, reps=1):
    """Build the Bass program.

    nsub: number of 128-query subtiles per core
    c:    candidate keypoints per subtile
    nrhs: number of rhs matrices (1 = shared by all subtiles, else nsub)
    g:    subtiles per output-staging group (nsub % g == 0)
    reps: repeat the whole computation (timing only)
    """
    assert nsub % g == 0
    n = nsub * 128
    nc = bacc.Bacc("TRN2", target_bir_lowering=False)
    lhs = nc.declare_dram_parameter("lhs", [4, n], F32, isOutput=False)
    rhs = nc.declare_dram_parameter("rhs", [nrhs, 4, c], F32, isOutput=False)
    a2d = nc.declare_dram_parameter("a2d", [nsub // g, 128, g], F32, isOutput=False)
    odist = nc.declare_dram_parameter("odist", [n, 8], F32, isOutput=True)
    oidx = nc.declare_dram_parameter("oidx", [n, 8], U32, isOutput=True)

    lc_sub = min(8, g)  # subtiles per lhs/rhs load chunk
    assert g % lc_sub == 0

    with tile.TileContext(nc) as tc, ExitStack() as ctx:
        if reps > 1:
            ctx.enter_context(tc.For_i(0, reps, 1))
        lpool = ctx.enter_context(tc.tile_pool(name="lhs", bufs=3))
        rpool = ctx.enter_context(tc.tile_pool(name="rhs", bufs=3))
        ppool = ctx.enter_context(tc.tile_pool(name="psum", bufs=6, space="PSUM"))
        dpool = ctx.enter_context(tc.tile_pool(name="d2", bufs=4))
        wpool = ctx.enter_context(tc.tile_pool(name="wide", bufs=2))

        rall = None
        if nrhs == 1:
            rall = rpool.tile([4, c], F32)
            nc.sync.dma_start(out=rall[:], in_=rhs[0])

        lc = None
        rc = None
        for m in range(nsub // g):
            wv = wpool.tile([128, g * 8], F32)
            wi = wpool.tile([128, g * 8], U32)
            a2t = wpool.tile([128, g], F32)
            nc.sync.dma_start(out=a2t[:], in_=a2d[m])
            for j in range(g):
                s = m * g + j
                if s % lc_sub == 0:
                    lc = lpool.tile([4, lc_sub * 128], F32)
                    nc.sync.dma_start(
                        out=lc[:], in_=lhs[:, s * 128 : (s + lc_sub) * 128]
                    )
                    if nrhs > 1:
                        rc = rpool.tile([4, lc_sub * c], F32)
                        nc.sync.dma_start(
                            out=rc[:].rearrange("f (s c) -> f s c", s=lc_sub),
                            in_=rhs[s : s + lc_sub].rearrange("s f c -> f s c"),
                        )
                jj = s % lc_sub
                lslice = lc[:, jj * 128 : (jj + 1) * 128]
                rslice = rall[:] if nrhs == 1 else rc[:, jj * c : (jj + 1) * c]
                if use_f32r:
                    lslice = lslice.bitcast(F32R)
                    rslice = rslice.bitcast(F32R)
                psum = ppool.tile([128, c], F32)
                nc.tensor.matmul(
                    psum[:], lhsT=lslice, rhs=rslice, start=True, stop=True
                )
                d2t = dpool.tile([128, c], F32)
                nc.scalar.copy(out=d2t[:], in_=psum[:])
                nc.vector.max(wv[:, j * 8 : (j + 1) * 8], d2t[:])
                nc.vector.max_index(
                    wi[:, j * 8 : (j + 1) * 8], wv[:, j * 8 : (j + 1) * 8], d2t[:]
                )
            # d2 = a2 - val (clamped to >= 1e-12), dist = sqrt(d2)
            a2x = wpool.tile([128, g * 8], F32)
            a2x3 = a2x[:].rearrange("p (g e) -> p g e", e=8)
            for e in range(8):
                nc.vector.tensor_copy(a2x3[:, :, e], a2t[:])
            d2n = wpool.tile([128, g * 8], F32)
            # d2n = val - a2 = -(d2);  min with -1e-12  ==  -(max(d2, 1e-12))
            nc.vector.tensor_sub(d2n[:], wv[:], a2x[:])
            nc.vector.tensor_scalar_min(d2n[:], d2n[:], -1e-12)
            dst = wpool.tile([128, g * 8], F32)
            nc.scalar.activation(
                dst[:], d2n[:], func=mybir.ActivationFunctionType.Sqrt, scale=-1.0
            )
            orows = odist[m * g * 128 : (m + 1) * g * 128].rearrange(
                "(p g) e -> p (g e)", p=128
            )
            nc.sync.dma_start(out=orows, in_=dst[:])
            irows = oidx[m * g * 128 : (m + 1) * g * 128].rearrange(
                "(p g) e -> p (g e)", p=128
            )
            nc.sync.dma_start(out=irows, in_=wi[:])
    nc.compile()
    return nc


# ---------------------------------------------------------------------------
# Host-side orchestration
# ---------------------------------------------------------------------------

_PROG_CACHE = {}


def _get_program(nsub, c, nrhs, g):
    key = (nsub, c, nrhs, g)
    if key not in _PROG_CACHE:
        _PROG_CACHE[key] = build_program(nsub, c, nrhs, g)
    return _PROG_CACHE[key]


def _core_inputs_v1(q, kp):
    """Simple dense config: one rhs of all 512 keypoints, identity order.

    q:  [65536, 3] float32 queries of this core
    kp: [512, 3] float32 keypoints of this core's batch
    Returns in_map dict. Device row r maps to query  (s*128 + p)  with
    s = (r // (128*g))*g + r % g,  p = (r // g) % 128.
    """
    nsub, c, g = NQ_CORE // 128, NKP, 64
    lhs = np.empty((4, NQ_CORE), np.float32)
    lhs[:3] = q.T
    lhs[3] = 1.0
    b2 = (kp[:, 0] * kp[:, 0] + kp[:, 1] * kp[:, 1]) + kp[:, 2] * kp[:, 2]
    rhs = np.empty((1, 4, c), np.float32)
    rhs[0, :3] = 2.0 * kp.T
    rhs[0, 3] = -b2
    a2 = (q[:, 0] * q[:, 0] + q[:, 1] * q[:, 1]) + q[:, 2] * q[:, 2]
    a2d = np.ascontiguousarray(
        a2.reshape(nsub // g, g, 128).transpose(0, 2, 1)
    )
    return {"lhs": lhs, "rhs": rhs, "a2d": a2d}


def _devrow_to_query(nsub, g):
    """origq[r] for device output row r (v1 ordering)."""
    r = np.arange(nsub * 128)
    m = r // (128 * g)
    j = r % g
    p = (r // g) % 128
    return (m * g + j) * 128 + p


def _devrow_to_slot(nsub, g):
    """slot (= s*128 + p) for device output row r."""
    r = np.arange(nsub * 128)
    m = r // (128 * g)
    j = r % g
    p = (r // g) % 128
    return (m * g + j) * 128 + p


# --- v2: spatial-cell candidate pruning ------------------------------------

D_GRID = 5
C_CAND = 80
G_V2 = 32


def _cell_candidates(kp, d, c):
    """Per-cell rhs [d^3+1, 4, c] and candidate id map [d^3+1, c].

    Cell cc covers box [i,j,l]/d..([i,j,l]+1)/d; candidates are keypoints
    within R+1e-3 of the box. Last row = dummy (all padded) for pad subtiles.
    Pad columns get [0,0,0,-1e30] so their score 2ab-b2 = -1e30 never wins.
    """
    ncell = d**3
    rhs = np.zeros((ncell + 1, 4, c), np.float32)
    rhs[:, 3, :] = -1e30
    cmap = np.zeros((ncell + 1, c), np.int32)
    kp64 = kp.astype(np.float64)
    side = 1.0 / d
    b2 = (kp[:, 0] * kp[:, 0] + kp[:, 1] * kp[:, 1]) + kp[:, 2] * kp[:, 2]
    thr = (R + 1e-3) ** 2
    for i in range(d):
        for j in range(d):
            for l in range(d):
                cc = (i * d + j) * d + l
                lo = np.array([i, j, l]) * side
                dd = np.maximum(np.maximum(lo - kp64, 0), kp64 - (lo + side))
                ids = np.nonzero((dd * dd).sum(1) < thr)[0]
                n = len(ids)
                assert n <= c, f"cell {cc}: {n} candidates > C={c}"
                rhs[cc, 0, :n] = 2.0 * kp[ids, 0]
                rhs[cc, 1, :n] = 2.0 * kp[ids, 1]
                rhs[cc, 2, :n] = 2.0 * kp[ids, 2]
                rhs[cc, 3, :n] = -b2[ids]
                cmap[cc, :n] = ids
    return rhs, cmap


def _assign_subtiles(q, d):
    """Sort queries into cells; chunk each cell into 128-query subtiles.

    Returns (perm2 [nsub_used*128] orig-query index with -1 padding,
             sub_cell [nsub_used] cell id per subtile).
    """
    nq = q.shape[0]
    cid = np.clip((q * d).astype(np.int32), 0, d - 1)
    cell = (cid[:, 0] * d + cid[:, 1]) * d + cid[:, 2]
    order = np.argsort(cell, kind="stable")
    counts = np.bincount(cell, minlength=d**3)
    perm2 = []
    sub_cell = []
    start = 0
    for cc in range(d**3):
        n = int(counts[cc])
        qs = order[start : start + n]
        start += n
        for o in range(0, n, 128):
            chunk = qs[o : o + 128]
            if len(chunk) < 128:
                chunk = np.concatenate(
                    [chunk, np.full(128 - len(chunk), -1, np.int64)]
                )
            perm2.append(chunk)
            sub_cell.append(cc)
    return np.concatenate(perm2), np.asarray(sub_cell, np.int64)


def _core_inputs_v2(q, kp, nsub, c, g):
    """Spatial config inputs + mappings for one core."""
    perm2, sub_cell = _assign_subtiles(q, D_GRID)
    nsub_used = len(sub_cell)
    assert nsub_used <= nsub, f"{nsub_used} subtiles > program NSUB={nsub}"
    perm2 = np.concatenate(
        [perm2, np.full((nsub - nsub_used) * 128, -1, np.int64)]
    )
    sub_cell = np.concatenate(
        [sub_cell, np.full(nsub - nsub_used, D_GRID**3, np.int64)]
    )

    cell_rhs, cell_cmap = _cell_candidates(kp, D_GRID, c)
    rhs = cell_rhs[sub_cell]  # [nsub, 4, c]
    cmap = cell_cmap[sub_cell]  # [nsub, c]

    qsafe = np.where(perm2 >= 0, perm2, 0)
    qc = q[qsafe]  # [nsub*128, 3] slot-ordered coords
    lhs = np.empty((4, nsub * 128), np.float32)
    lhs[:3] = qc.T
    lhs[3] = 1.0
    a2 = (qc[:, 0] * qc[:, 0] + qc[:, 1] * qc[:, 1]) + qc[:, 2] * qc[:, 2]
    a2d = np.ascontiguousarray(a2.reshape(nsub // g, g, 128).transpose(0, 2, 1))
    return {"lhs": lhs, "rhs": rhs, "a2d": a2d}, perm2, cmap


def kernel_v2(x, kp_pos):
    x = np.asarray(x, dtype=np.float32)
    kp_pos = np.asarray(kp_pos, dtype=np.float32)
    rays = T * RO
    c, g = C_CAND, G_V2

    xq = x.reshape(B, 2, rays // 2 * S, 3)
    preps = []
    nsub_needed = 0
    for core in range(N_CORES):
        b, half = divmod(core, 2)
        perm2, sub_cell = _assign_subtiles(xq[b, half], D_GRID)
        nsub_needed = max(nsub_needed, len(sub_cell))
    nsub = ((nsub_needed + g - 1) // g) * g

    in_maps, perms, cmaps = [], [], []
    for core in range(N_CORES):
        b, half = divmod(core, 2)
        im, perm2, cmap = _core_inputs_v2(xq[b, half], kp_pos[b], nsub, c, g)
        in_maps.append(im)
        perms.append(perm2)
        cmaps.append(cmap)

    nc = _get_program(nsub, c, nsub, g)
    res = run_bass_kernel_spmd(nc, in_maps, core_ids=list(range(N_CORES)))

    slot_of_devrow = _devrow_to_slot(nsub, g)
    devrow_of_slot = np.empty_like(slot_of_devrow)
    devrow_of_slot[slot_of_devrow] = np.arange(slot_of_devrow.size)

    vals = np.empty((B, rays * S, 8), np.float32)
    idx = np.empty((B, rays * S, 8), np.int32)
    half_n = rays // 2 * S
    s_of_slot = np.arange(nsub * 128) // 128
    for core in range(N_CORES):
        b, half = divmod(core, 2)
        od = res.results[core]["odist"][devrow_of_slot]  # slot order
        oi = res.results[core]["oidx"].view(np.int32)[devrow_of_slot]
        kpid = cmaps[core][s_of_slot[:, None], oi]  # [n, 8]
        perm2 = perms[core]
        valid = perm2 >= 0
        dst = vals[b, half * half_n : (half + 1) * half_n]
        dsti = idx[b, half * half_n : (half + 1) * half_n]
        dst[perm2[valid]] = od[valid]
        dsti[perm2[valid]] = kpid[valid]

    return _postprocess(x, vals, idx)


def kernel(x, kp_pos):
    x = np.asarray(x, dtype=np.float32)
    kp_pos = np.asarray(kp_pos, dtype=np.float32)
    rays = T * RO

    nsub, c, nrhs, g = NQ_CORE // 128, NKP, 1, 64
    nc = _get_program(nsub, c, nrhs, g)

    xq = x.reshape(B, 2, rays // 2 * S, 3)
    in_maps = []
    for core in range(N_CORES):
        b, half = divmod(core, 2)
        in_maps.append(_core_inputs_v1(xq[b, half], kp_pos[b]))

    res = run_bass_kernel_spmd(nc, in_maps, core_ids=list(range(N_CORES)))

    perm = _devrow_to_query(nsub, g)
    inv = np.empty_like(perm)
    inv[perm] = np.arange(perm.size)

    vals = np.empty((B, rays * S, 8), np.float32)
    idx = np.empty((B, rays * S, 8), np.int32)
    half_n = rays // 2 * S
    for core in range(N_CORES):
        b, half = divmod(core, 2)
        od = res.results[core]["odist"]
        oi = res.results[core]["oidx"].view(np.int32)
        vals[b, half * half_n : (half + 1) * half_n] = od[inv]
        idx[b, half * half_n : (half + 1) * half_n] = oi[inv]

    return _postprocess(x, vals, idx)


def _postprocess(x, vals, idx):
    """vals: [B, rays*S, 8] ascending distances; idx: keypoint ids (0..511)."""
    rays = T * RO
    vals = vals.reshape(B, rays, S, 8)
    idx = idx.reshape(B, rays, S, 8)

    valid_nb = vals < R
    offset = (NKP * np.arange(B, dtype=np.int32)).reshape(B, 1, 1, 1)
    nb_idx = np.where(valid_nb, idx + offset, -1).astype(np.int32)

    valid_pts = valid_nb[..., 0:1]  # any() == slot 0 since ascending
    csum = np.cumsum(valid_pts.astype(np.int32), axis=-2)
    valid_pts = np.logical_and(valid_pts, csum <= MAX_SHADING_PTS)

    nb_idx = np.where(valid_pts, nb_idx, -1)
    nb_dist = np.where(np.logical_and(valid_pts, valid_nb), vals, 0.0).astype(
        np.float32
    )
    shading = np.where(valid_pts, x.reshape(B, rays, S, 3), 0.0).astype(np.float32)

    num_valid = valid_pts.sum(axis=-2, keepdims=True)
    mask = np.arange(MAX_SHADING_PTS).reshape(1, 1, -1, 1) < num_valid

    return (
        nb_idx.reshape(B, T, RO, S, K),
        shading.reshape(B, T, RO, S, 3),
        nb_dist.reshape(B, T, RO, S, K),
        mask.reshape(B, T, RO, MAX_SHADING_PTS, 1),
    )
